# revision 8
# baseline (speedup 1.0000x reference)
"""Trainium2 Bass kernel for nn_CVRP_Encoder (AFT-style CVRP encoder).

Strategy: data-parallel over batch B=32 across 8 NeuronCores (4 items/core).
Per item, everything is kept in a transposed [D=128 (partitions), S=1000]
layout so instance-norm reduces along the free axis. The S axis is split
into 8 chunks of 125 for the attention contraction (t on partitions) and
2 chunks of 500 for matmul free dims.

Self-contained: hardcodes all shapes; host-side numpy does only layout
prep (transposes/casts/shard) + final gather.
"""
import sys

sys.path.insert(0, "/opt/trn_rl_repo")

import numpy as np

import concourse.bass as bass
import concourse.tile as tile
from concourse import bacc, mybir
from concourse.bass_utils import run_bass_kernel_spmd

F32 = mybir.dt.float32
F16 = mybir.dt.float16
BF16 = mybir.dt.bfloat16
I32 = mybir.dt.int32
AF = mybir.ActivationFunctionType
ALU = mybir.AluOpType

# problem shapes
B, N, D, F, L = 32, 999, 128, 512, 6
S = N + 1            # 1000
P = 128
NCORES = 8
IPC = B // NCORES    # 4 items per core
TC = 8               # t-chunks
TCS = S // TC        # 125 per t-chunk
SC = 2               # s-chunks (matmul free dim)
SCS = S // SC        # 500
FC = F // P          # 4 f-chunks
EPS = 1e-5
RSQRT_MAGIC = 0x5F3759DF + 1


def _bcast_dram(handle, n_part, idx, count):
    """AP reading DRAM vector handle[idx:idx+count] broadcast across n_part partitions."""
    ap = handle[:]
    return bass.AP(tensor=ap.tensor, offset=idx, ap=[[0, n_part], [1, count]])


def _nv(t):
    """[P, 1024] tile/psum -> [P, 2, 500] strided view (skip 512-alignment pad)."""
    return t[:].rearrange("p (n s) -> p n s", n=2)[:, :, 0:SCS]


def build_cvrp(cs):
    """cs: list of L per-layer scale constants c_l = log_scale * alpha[l]."""
    shared_es = all(abs(c - cs[0]) < 1e-30 for c in cs)

    nc = bacc.Bacc("TRN2", target_bir_lowering=False, debug=False,
                   num_devices=NCORES)

    g = {}
    g["dist_t"] = nc.declare_dram_parameter("dist_t", [IPC, TC, TCS, S], BF16, isOutput=False)
    g["node_t"] = nc.declare_dram_parameter("node_t", [IPC, 3, N], F32, isOutput=False)
    g["depot"] = nc.declare_dram_parameter("depot", [IPC, 2], F32, isOutput=False)
    g["flagf"] = nc.declare_dram_parameter("flagf", [IPC], F32, isOutput=False)
    g["wqt"] = nc.declare_dram_parameter("wqt", [L, D, D], F16, isOutput=False)
    g["wkt"] = nc.declare_dram_parameter("wkt", [L, D, D], F16, isOutput=False)
    g["wvt"] = nc.declare_dram_parameter("wvt", [L, D, D], F16, isOutput=False)
    g["w1t"] = nc.declare_dram_parameter("w1t", [L, D, F], F16, isOutput=False)
    g["w2t"] = nc.declare_dram_parameter("w2t", [L, P, FC, D], F16, isOutput=False)
    g["wnt"] = nc.declare_dram_parameter("wnt", [3, D], F32, isOutput=False)
    g["wdt"] = nc.declare_dram_parameter("wdt", [2, D], F32, isOutput=False)
    g["wint"] = nc.declare_dram_parameter("wint", [D, D], F32, isOutput=False)
    g["woutt"] = nc.declare_dram_parameter("woutt", [D, D], F32, isOutput=False)
    g["biases4"] = nc.declare_dram_parameter("biases4", [D, 4], F32, isOutput=False)
    g["bw1_t"] = nc.declare_dram_parameter("bw1_t", [D, L, FC], F32, isOutput=False)
    g["bw2_t"] = nc.declare_dram_parameter("bw2_t", [D, L], F32, isOutput=False)
    g["g1_t"] = nc.declare_dram_parameter("g1_t", [D, L], F32, isOutput=False)
    g["b1_t"] = nc.declare_dram_parameter("b1_t", [D, L], F32, isOutput=False)
    g["g2_t"] = nc.declare_dram_parameter("g2_t", [D, L], F32, isOutput=False)
    g["b2_t"] = nc.declare_dram_parameter("b2_t", [D, L], F32, isOutput=False)
    g["out32"] = nc.declare_dram_parameter("out32", [IPC, D, S], F32, isOutput=True)

    with tile.TileContext(nc) as tc_ctx:
        _body(nc, tc_ctx, g, cs, shared_es)
    nc.compile()
    return nc


def _norm_smalls(nc, np_, sums, sumsq, g_col, b_col, tag):
    """Batched instance-norm scalar math on [D, IPC] tiles.
    Returns (A, C) [D, IPC] with out = A*y + C per item column."""
    sm = np_.tile([D, 8, IPC], F32, tag=f"nsm_{tag}")
    mean, msq, var = sm[:, 0], sm[:, 1], sm[:, 2]
    nc.vector.tensor_scalar(mean, sums, 1.0 / S, None, ALU.mult)
    nc.vector.tensor_tensor(msq, mean, mean, ALU.mult)
    nc.vector.tensor_scalar(var, sumsq, 1.0 / S, EPS, ALU.mult, ALU.add)
    nc.vector.tensor_tensor(var, var, msq, ALU.subtract)  # biased var + eps
    ry = sm[:, 3]
    ibits = ry.bitcast(I32)
    nc.vector.tensor_scalar(ibits, var.bitcast(I32), 1, -1,
                            ALU.logical_shift_right, ALU.bitwise_xor)
    nc.vector.tensor_scalar(ibits, ibits, RSQRT_MAGIC, None, ALU.add)
    t1, t2 = sm[:, 4], sm[:, 5]
    for _ in range(2):
        nc.vector.tensor_tensor(t1, ry, ry, ALU.mult)
        nc.vector.tensor_tensor(t2, t1, var, ALU.mult)
        nc.vector.tensor_scalar(t2, t2, -0.5, 1.5, ALU.mult, ALU.add)
        nc.vector.tensor_tensor(ry, ry, t2, ALU.mult)
    A, C = sm[:, 6], sm[:, 7]
    nc.vector.tensor_scalar(A, ry, g_col, None, ALU.mult)       # rstd * g  (g bcast)
    nc.vector.tensor_tensor(C, mean, A, ALU.mult)
    nc.vector.tensor_scalar(C, C, b_col, -1.0, ALU.subtract, ALU.mult)  # (mean*A - b) * -1
    return A, C


def _body(nc, tc, g, cs, shared_es):
    from contextlib import ExitStack

    ctx = ExitStack()
    singles = ctx.enter_context(tc.tile_pool(name="singles", bufs=1))
    xpool = ctx.enter_context(tc.tile_pool(name="xpool", bufs=1))
    tp = ctx.enter_context(tc.tile_pool(name="tp", bufs=2))
    scr = ctx.enter_context(tc.tile_pool(name="scr", bufs=2))
    np_ = ctx.enter_context(tc.tile_pool(name="npool", bufs=2))
    pp = ctx.enter_context(tc.tile_pool(name="pp", bufs=1))
    ps = ctx.enter_context(tc.tile_pool(name="ps", bufs=4, space="PSUM"))

    # ---- resident weights ----
    t_wqt, t_wkt, t_wvt, t_w1t, t_w2t = [], [], [], [], []
    for l in range(L):
        for lst, src, shape in ((t_wqt, g["wqt"], [D, D]), (t_wkt, g["wkt"], [D, D]),
                                (t_wvt, g["wvt"], [D, D]), (t_w1t, g["w1t"], [D, F]),
                                (t_w2t, g["w2t"], [P, FC, D])):
            w = singles.tile(shape, F16, tag=f"w{id(lst)}_{l}")
            nc.sync.dma_start(w[:], src[l])
            lst.append(w)
    t_wnt = singles.tile([3, D], F32, tag="wnt")
    nc.sync.dma_start(t_wnt[:], g["wnt"][:])
    t_wdt = singles.tile([2, D], F32, tag="wdt")
    nc.sync.dma_start(t_wdt[:], g["wdt"][:])
    t_wint = singles.tile([D, D], F32, tag="wint")
    nc.sync.dma_start(t_wint[:], g["wint"][:])
    t_woutt = singles.tile([D, D], F32, tag="woutt")
    nc.sync.dma_start(t_woutt[:], g["woutt"][:])
    small_names = ["biases4", "bw1_t", "bw2_t", "g1_t", "b1_t", "g2_t", "b2_t"]
    small_shapes = [[D, 4], [D, L, FC], [D, L], [D, L], [D, L], [D, L], [D, L]]
    sm_t = {}
    for nm, shp in zip(small_names, small_shapes):
        t = singles.tile(shp, F32, tag=nm)
        nc.sync.dma_start(t[:], g[nm][:])
        sm_t[nm] = t
    t_b4, t_bw1, t_bw2 = sm_t["biases4"], sm_t["bw1_t"], sm_t["bw2_t"]
    t_g1, t_b1, t_g2, t_b2 = sm_t["g1_t"], sm_t["b1_t"], sm_t["g2_t"], sm_t["b2_t"]
    t_ff = singles.tile([P, IPC], F32, tag="ffl")
    nc.sync.dma_start(t_ff[:], _bcast_dram(g["flagf"], P, 0, IPC))

    BD, BN_, BIN, BOUT = (t_b4[:, i : i + 1] for i in range(4))

    # ---- embedding (fp32) ----
    x32s, x16s = [], []
    for i in range(IPC):
        x32 = xpool.tile([D, S], F32, tag=f"x32_{i}")
        t_node = tp.tile([3, N], F32, tag="node")
        nc.sync.dma_start(t_node[:], g["node_t"][i])
        t_dep = tp.tile([2, 1], F32, tag="dep")
        nc.sync.dma_start(t_dep[:], g["depot"][i, :, None])
        pe = ps.tile([P, 1024], F32, tag="ps")
        nc.tensor.matmul(pe[:, 0:500], t_wnt[:], t_node[:, 0:500], start=True, stop=True)
        nc.tensor.matmul(pe[:, 512:1011], t_wnt[:], t_node[:, 500:999], start=True, stop=True)
        nc.scalar.activation(x32[:, 1:501], pe[:, 0:500], AF.Identity, bias=BN_, scale=1.0)
        nc.scalar.activation(x32[:, 501:1000], pe[:, 512:1011], AF.Identity, bias=BN_, scale=1.0)
        pd = ps.tile([P, 1024], F32, tag="ps")
        nc.tensor.matmul(pd[:, 0:1], t_wdt[:], t_dep[:], start=True, stop=True)
        nc.scalar.activation(x32[:, 0:1], pd[:, 0:1], AF.Identity, bias=BD, scale=1.0)
        pw = ps.tile([P, 1024], F32, tag="ps")
        nc.tensor.matmul(pw[:, 0:1], t_wint[:], x32[:, 1:2], start=True, stop=True)
        nc.scalar.activation(x32[:, 1:2], pw[:, 0:1], AF.Identity, bias=BIN, scale=1.0)
        # flag row fix: u = f*x0 + (1-f)*x999 ; w = Wout@u + bout ;
        # x0 += f*(w-u) ; x999 += (1-f)*(w-u)
        fcol = t_ff[:, i : i + 1]
        sm = np_.tile([D, 8], F32, tag="flagtmp")
        d1, u, t2, w_sb, d0 = (sm[:, j : j + 1] for j in range(5))
        nc.vector.tensor_tensor(d1, x32[:, 0:1], x32[:, 999:1000], ALU.subtract)
        nc.vector.tensor_scalar(d1, d1, fcol, None, ALU.mult)
        nc.vector.tensor_tensor(u, x32[:, 999:1000], d1, ALU.add)
        pf = ps.tile([P, 1024], F32, tag="ps")
        nc.tensor.matmul(pf[:, 0:1], t_woutt[:], u, start=True, stop=True)
        nc.scalar.activation(w_sb, pf[:, 0:1], AF.Identity, bias=BOUT, scale=1.0)
        nc.vector.tensor_tensor(t2, w_sb, u, ALU.subtract)          # w - u
        nc.vector.tensor_scalar(d0, t2, fcol, None, ALU.mult)       # f*(w-u)
        nc.vector.tensor_tensor(x32[:, 0:1], x32[:, 0:1], d0, ALU.add)
        nc.vector.tensor_tensor(x32[:, 999:1000], x32[:, 999:1000], t2, ALU.add)
        nc.vector.tensor_tensor(x32[:, 999:1000], x32[:, 999:1000], d0, ALU.subtract)
        x16 = xpool.tile([D, S], F16, tag=f"x16_{i}")
        nc.vector.tensor_copy(x16[:], x32[:])
        x32s.append(x32)
        x16s.append(x16)

    # ---- es (attention kernel matrix), fp16 resident ----
    es_tiles = []
    if shared_es:
        for i in range(IPC):
            es = singles.tile([P, TC, S], BF16, tag=f"es{i}")
            nc.sync.dma_start(es[:TCS], g["dist_t"][i].rearrange("c p s -> p c s"))
            nc.scalar.activation(es[:TCS], es[:TCS], AF.Exp, bias=0.0, scale=-cs[0])
            es_tiles.append(es)

    # ---- encoder layers ----
    for l in range(L):
        ys, y2s, h16s = [None] * IPC, [None] * IPC, [None] * IPC
        st1 = np_.tile([D, 2, IPC], F32, tag="st1")
        st2 = np_.tile([D, 2, IPC], F32, tag="st2")
        for i in range(IPC):
            x32, x16 = x32s[i], x16s[i]
            pq = ps.tile([P, 1024], F32, tag="ps")
            nc.tensor.matmul(pq[:, 0:500], t_wqt[l][:], x16[:, 0:500], start=True, stop=True)
            nc.tensor.matmul(pq[:, 512:1012], t_wqt[l][:], x16[:, 500:1000], start=True, stop=True)
            pk = ps.tile([P, 1024], F32, tag="ps")
            pv = ps.tile([P, 1024], F32, tag="ps")
            for c in range(TC):
                lhs = x16[:, c * TCS : (c + 1) * TCS]
                nc.tensor.matmul(pk[:TCS, c * P : (c + 1) * P], lhs, t_wkt[l][:], start=True, stop=True)
                nc.tensor.matmul(pv[:TCS, c * P : (c + 1) * P], lhs, t_wvt[l][:], start=True, stop=True)
            tq = tp.tile([P, S], F16, tag="tq")
            nc.scalar.activation(tq[:].rearrange("p (n s) -> p n s", n=2), _nv(pq),
                                 AF.Tanh, bias=0.0, scale=0.5)
            nc.vector.tensor_scalar(tq[:], tq[:], 0.5, 0.5, ALU.mult, ALU.add)
            ek = tp.tile([P, TC * P], BF16, tag="ek")
            nc.scalar.activation(ek[:TCS], pk[:TCS], AF.Exp, bias=0.0, scale=1.0)
            ekv = tp.tile([P, TC * P], BF16, tag="ekv")
            nc.vector.tensor_tensor(ekv[:TCS], ek[:TCS], pv[:TCS], ALU.mult)
            if shared_es:
                es = es_tiles[i]
            else:
                es = tp.tile([P, TC, S], BF16, tag="es_dyn")
                nc.sync.dma_start(es[:TCS], g["dist_t"][i].rearrange("c p s -> p c s"))
                nc.scalar.activation(es[:TCS], es[:TCS], AF.Exp, bias=0.0, scale=-cs[l])
            pnum = ps.tile([P, 1024], F32, tag="ps")
            pden = ps.tile([P, 1024], F32, tag="ps")
            for sc in range(SC):
                off = sc * 512
                ssl = slice(sc * SCS, (sc + 1) * SCS)
                for c in range(TC):
                    nc.tensor.matmul(pnum[:, off : off + SCS],
                                     ekv[:TCS, c * P : (c + 1) * P], es[:TCS, c, ssl],
                                     start=(c == 0), stop=(c == TC - 1))
                for c in range(TC):
                    nc.tensor.matmul(pden[:, off : off + SCS],
                                     ek[:TCS, c * P : (c + 1) * P], es[:TCS, c, ssl],
                                     start=(c == 0), stop=(c == TC - 1))
            rden = scr.tile([P, 1024], F32, tag="scr4k")
            nc.vector.reciprocal_approx_fast(out=_nv(rden), in_=_nv(pden))
            wgt = tp.tile([P, S], F16, tag="wgt")
            nc.vector.tensor_tensor(wgt[:].rearrange("p (n s) -> p n s", n=2),
                                    _nv(pnum), _nv(rden), ALU.mult)
            nc.vector.tensor_tensor(wgt[:], wgt[:], tq[:], ALU.mult)
            y = pp.tile([P, S], F32, tag=f"y_{i}")
            nc.vector.tensor_tensor(y[:], x32[:], wgt[:], ALU.add)
            ys[i] = y
            nc.vector.tensor_reduce(st1[:, 0, i : i + 1], y[:], axis=mybir.AxisListType.X, op=ALU.add)
            sq = scr.tile([P, 1024], F32, tag="scr4k")
            nc.scalar.activation(sq[:, 0:S], y[:], AF.Square, accum_out=st1[:, 1, i : i + 1])
        A1, C1 = _norm_smalls(nc, np_, st1[:, 0], st1[:, 1],
                              t_g1[:, l : l + 1], t_b1[:, l : l + 1], "n1")
        for i in range(IPC):
            h16 = pp.tile([P, S], F16, tag=f"h16_{i}")
            nc.vector.tensor_scalar(h16[:], ys[i][:], A1[:, i : i + 1], C1[:, i : i + 1],
                                    ALU.mult, ALU.add)
            h32 = pp.tile([P, S], F32, tag=f"h32_{i}")
            nc.vector.tensor_scalar(h32[:], ys[i][:], A1[:, i : i + 1], C1[:, i : i + 1],
                                    ALU.mult, ALU.add)
            h16s[i] = h16
            pf2 = ps.tile([P, 1024], F32, tag="ps")
            for fc in range(FC):
                pf1 = ps.tile([P, 1024], F32, tag="ps")
                w1 = t_w1t[l][:, fc * P : (fc + 1) * P]
                nc.tensor.matmul(pf1[:, 0:500], w1, h16[:, 0:500], start=True, stop=True)
                nc.tensor.matmul(pf1[:, 512:1012], w1, h16[:, 500:1000], start=True, stop=True)
                r16 = tp.tile([P, S], F16, tag="r16")
                nc.scalar.activation(r16[:].rearrange("p (n s) -> p n s", n=2), _nv(pf1),
                                     AF.Relu, bias=t_bw1[:, l, fc : fc + 1], scale=1.0)
                for sc in range(SC):
                    nc.tensor.matmul(pf2[:, sc * 512 : sc * 512 + SCS],
                                     t_w2t[l][:, fc, :], r16[:, sc * SCS : (sc + 1) * SCS],
                                     start=(fc == 0), stop=(fc == FC - 1))
            y2 = pp.tile([P, S], F32, tag=f"y_{i}")
            nc.scalar.activation(y2[:].rearrange("p (n s) -> p n s", n=2), _nv(pf2),
                                 AF.Identity, bias=t_bw2[:, l : l + 1], scale=1.0)
            nc.vector.tensor_tensor(y2[:], y2[:], h32[:], ALU.add)
            y2s[i] = y2
            nc.vector.tensor_reduce(st2[:, 0, i : i + 1], y2[:], axis=mybir.AxisListType.X, op=ALU.add)
            sq = scr.tile([P, 1024], F32, tag="scr4k")
            nc.scalar.activation(sq[:, 0:S], y2[:], AF.Square, accum_out=st2[:, 1, i : i + 1])
        A2, C2 = _norm_smalls(nc, np_, st2[:, 0], st2[:, 1],
                              t_g2[:, l : l + 1], t_b2[:, l : l + 1], "n2")
        for i in range(IPC):
            gen = "b" if l % 2 == 0 else ""
            nx32 = xpool.tile([D, S], F32, tag=f"x32_{i}{gen}")
            nc.vector.tensor_scalar(nx32[:], y2s[i][:], A2[:, i : i + 1], C2[:, i : i + 1],
                                    ALU.mult, ALU.add)
            nx16 = None
            if l < L - 1:
                nx16 = xpool.tile([D, S], F16, tag=f"x16_{i}{gen}")
                nc.vector.tensor_scalar(nx16[:], y2s[i][:], A2[:, i : i + 1], C2[:, i : i + 1],
                                        ALU.mult, ALU.add)
            x32s[i], x16s[i] = nx32, nx16

    for i in range(IPC):
        nc.sync.dma_start(g["out32"][i], x32s[i][:])

    ctx.close()


# ------------------------------------------------------------------
# host wrapper
# ------------------------------------------------------------------
_cache = {}


def _get_nc(cs_key):
    if cs_key not in _cache:
        _cache[cs_key] = build_cvrp(list(cs_key))
    return _cache[cs_key]


def prep_inputs(depot_xy, node_xy_demand, dist, log_scale, flag,
                Wd, bd, Wn, bn, Win, bin_, Wout, bout,
                Wq, Wk, Wv, alpha, g1, b1, W1, bW1, W2, bW2, g2, b2):
    """Host-side layout prep. Returns (cs, in_maps)."""
    dist = np.asarray(dist)
    flag = np.asarray(flag)
    cs = tuple(float(np.asarray(log_scale)[0]) * float(a) for a in np.asarray(alpha))

    import ml_dtypes
    dist_t = np.ascontiguousarray(np.asarray(dist).transpose(0, 2, 1)).astype(ml_dtypes.bfloat16)
    dist_t = dist_t.reshape(B, TC, TCS, S)
    node_t = np.ascontiguousarray(np.asarray(node_xy_demand).transpose(0, 2, 1)).astype(np.float32)
    depot = np.asarray(depot_xy).reshape(B, 2).astype(np.float32)
    flagf = flag.astype(np.float32)

    f16 = lambda a: np.ascontiguousarray(np.asarray(a)).astype(np.float16)
    f32 = lambda a: np.ascontiguousarray(np.asarray(a)).astype(np.float32)
    shared = {
        "wqt": f16(np.asarray(Wq).transpose(0, 2, 1)),
        "wkt": f16(np.asarray(Wk).transpose(0, 2, 1)),
        "wvt": f16(np.asarray(Wv).transpose(0, 2, 1)),
        "w1t": f16(np.asarray(W1).transpose(0, 2, 1)),
        "w2t": f16(np.asarray(W2).transpose(0, 2, 1).reshape(L, FC, P, D).transpose(0, 2, 1, 3)),
        "wnt": f32(np.asarray(Wn).T),
        "wdt": f32(np.asarray(Wd).T),
        "wint": f32(np.asarray(Win).T),
        "woutt": f32(np.asarray(Wout).T),
        "biases4": f32(np.stack([np.asarray(bd), np.asarray(bn),
                                 np.asarray(bin_), np.asarray(bout)], axis=1)),
        "bw1_t": f32(np.asarray(bW1).reshape(L, FC, P).transpose(2, 0, 1)),
        "bw2_t": f32(np.asarray(bW2).T),
        "g1_t": f32(np.asarray(g1).T),
        "b1_t": f32(np.asarray(b1).T),
        "g2_t": f32(np.asarray(g2).T),
        "b2_t": f32(np.asarray(b2).T),
    }
    in_maps = []
    for c in range(NCORES):
        sl = slice(c * IPC, (c + 1) * IPC)
        m = dict(shared)
        m["dist_t"] = dist_t[sl]
        m["node_t"] = node_t[sl]
        m["depot"] = depot[sl]
        m["flagf"] = flagf[sl]
        in_maps.append(m)
    return cs, in_maps


TRACE = False
LAST_RESULT = None


def kernel(**inputs):
    global LAST_RESULT
    cs, in_maps = prep_inputs(**inputs)
    nc = _get_nc(cs)
    res = run_bass_kernel_spmd(nc, in_maps, list(range(NCORES)), trace=TRACE)
    LAST_RESULT = res
    out = np.concatenate([r["out32"] for r in res.results], axis=0)  # [B, D, S]
    return np.ascontiguousarray(out.transpose(0, 2, 1)).astype(np.float32)
